# revision 15
# baseline (speedup 1.0000x reference)
"""KL-attention kernel for Trainium2, 8-core data-parallel over batch.

Math (per batch b, x = [N=1024, D=1024] fp32):
  p = softmax(x, -1); logp = log_softmax(x, -1)
  S[i,j] = sum_d p[i,d] logp[j,d]         (attn = softmax(S, -1): neg_ent row
                                           offset cancels in the row softmax)
  Using sum_d p[i,d] = 1:  S[i,j] = (p @ x^T)[i,j] - logZ[j]
  out = softmax(S, -1) @ x

Implementation per batch (tiles of 128 rows, T = 8 tiles):
  E = exp(x) with per-row accumulate -> Z          (ACT, one pass)
  pT = (E^T) * diag(1/Z) via PE matmul against diag(1/Z)  (transpose + softmax
       normalization fused into one matmul)
  xT via PE matmul against identity
  S^T[j,i] = sum_d xT[d,j] pT[d,i]                 (PE, fp32 PSUM)
  esT = exp(S^T + (-logZ[j]))                      (ACT from PSUM, per-partition
                                                    bias, bf16 out; no row-max
                                                    needed: S in [-13, -2])
  U[i,d] = sum_j esT[j,i] x[j,d]; z[i] = sum_j esT[j,i]  (PE, ones column)
  out = U * (1/z[i])                               (DVE per-partition scale)

All matmul operands bf16 (fp32 PSUM accumulation); measured global rel err
~2.5e-3 vs the fp32 reference.
"""

import os

import numpy as np

try:
    import concourse.bass as bass  # noqa: F401
except ImportError:
    import sys

    sys.path.insert(0, "/opt/trn_rl_repo")

from contextlib import ExitStack

import concourse.bass as bass
import concourse.mybir as mybir
import concourse.tile as tile
from concourse import bacc
from concourse.bass_utils import run_bass_kernel_spmd
from concourse.masks import make_identity

F32 = mybir.dt.float32
BF16 = mybir.dt.bfloat16
AF = mybir.ActivationFunctionType

N_CORES = 8
B_PER_CORE = int(os.environ.get("KL_BPC", "4"))
N = 1024
D = 1024
P = 128
T = N // P  # 8 row tiles
XB_STRIDE = D + 8  # bf16 x tile row: 1024 data + 1 ones col + 7 pad


def build_kernel_body(ctx: ExitStack, tc: "tile.TileContext", x_ap, out_ap):
    nc = tc.nc
    STAGE = int(os.environ.get("KL_STAGE", "99"))

    consts = ctx.enter_context(tc.tile_pool(name="consts", bufs=1))
    xfpool = ctx.enter_context(tc.tile_pool(name="xf", bufs=1))
    xbpool = ctx.enter_context(tc.tile_pool(name="xb", bufs=2))
    ebpool = ctx.enter_context(tc.tile_pool(name="eb", bufs=1))
    dgpool = ctx.enter_context(tc.tile_pool(name="dg", bufs=2))
    xtpool = ctx.enter_context(tc.tile_pool(name="xt", bufs=1))
    ptpool = ctx.enter_context(tc.tile_pool(name="pt", bufs=1))
    espool = ctx.enter_context(tc.tile_pool(name="es", bufs=2))
    outpool = ctx.enter_context(tc.tile_pool(name="of", bufs=3))
    stats = ctx.enter_context(tc.tile_pool(name="st", bufs=2))
    mmpsum = ctx.enter_context(tc.tile_pool(name="mmps", bufs=4, space="PSUM"))

    ident_f = consts.tile([P, P], F32)
    make_identity(nc, ident_f[:, :])
    ident = consts.tile([P, P], BF16)
    nc.vector.tensor_copy(ident[:, :], ident_f[:, :])

    for b in range(B_PER_CORE):
        # ---- load + row stats ----
        xf = xfpool.tile([P, T * D], F32, tag="xf")
        for t in range(T):
            nc.sync.dma_start(
                xf[:, t * D : (t + 1) * D], x_ap[b, t * P : (t + 1) * P, :]
            )
        if STAGE < 1:
            continue
        xb = xbpool.tile([P, T * XB_STRIDE], BF16, tag="xb")
        eb = ebpool.tile([P, T * D], BF16, tag="eb")
        zs = stats.tile([P, T], F32, tag="zs")
        for t in range(T):
            nc.scalar.activation(
                eb[:, t * D : (t + 1) * D],
                xf[:, t * D : (t + 1) * D],
                AF.Exp,
                accum_out=zs[:, t : t + 1],
            )
            nc.vector.tensor_copy(
                xb[:, t * XB_STRIDE : t * XB_STRIDE + D],
                xf[:, t * D : (t + 1) * D],
            )
        # ones columns (8 per tile) for the second-softmax normalizer
        for t in range(T):
            nc.gpsimd.memset(
                xb[:, t * XB_STRIDE + D : t * XB_STRIDE + D + 8], 1.0
            )

        rz = stats.tile([P, T], F32, tag="rz")
        nlz = stats.tile([P, T], F32, tag="nlz")
        nc.vector.reciprocal(rz[:, :], zs[:, :])
        nc.scalar.activation(nlz[:, :], rz[:, :], AF.Ln)  # -log(Z)

        dg = dgpool.tile([P, T * P], BF16, tag="dg")
        for t in range(T):
            nc.vector.tensor_scalar_mul(
                dg[:, t * P : (t + 1) * P], ident[:, :], rz[:, t : t + 1]
            )

        # ---- transposes: xT (vs identity) and pT (vs diag(1/Z)) ----
        if STAGE < 2:
            continue
        xt = xtpool.tile([P, T * D], BF16, tag="xt")
        pt = ptpool.tile([P, T * D], BF16, tag="pt")
        for k in range(T):
            ps_x = mmpsum.tile([P, D], F32, tag="ps")
            for t in range(T):
                nc.tensor.matmul(
                    ps_x[:, t * P : (t + 1) * P],
                    xb[:, t * XB_STRIDE + k * P : t * XB_STRIDE + (k + 1) * P],
                    ident[:, :],
                    start=True,
                    stop=True,
                )
            nc.vector.tensor_copy(xt[:, k * D : (k + 1) * D], ps_x[:, :])
            ps_p = mmpsum.tile([P, D], F32, tag="ps")
            for t in range(T):
                nc.tensor.matmul(
                    ps_p[:, t * P : (t + 1) * P],
                    eb[:, t * D + k * P : t * D + (k + 1) * P],
                    dg[:, t * P : (t + 1) * P],
                    start=True,
                    stop=True,
                )
            nc.scalar.copy(pt[:, k * D : (k + 1) * D], ps_p[:, :])

        # ---- MM1: S^T[j,:] then exp(+bias) ----
        if STAGE < 3:
            continue
        est = espool.tile([P, T * D], BF16, tag="es")
        for j in range(T):
            ps_s = mmpsum.tile([P, D], F32, tag="ps")
            for c in range(2):
                for d in range(T):
                    nc.tensor.matmul(
                        ps_s[:, c * 512 : (c + 1) * 512],
                        xt[:, d * D + j * P : d * D + (j + 1) * P],
                        pt[:, d * D + c * 512 : d * D + (c + 1) * 512],
                        start=(d == 0),
                        stop=(d == T - 1),
                    )
            nc.scalar.activation(
                est[:, j * D : (j + 1) * D],
                ps_s[:, :],
                AF.Exp,
                bias=nlz[:, j : j + 1],
            )

        # ---- MM2: U = esT^T @ x, z = esT^T @ 1, out = U/z ----
        if STAGE < 4:
            continue
        for i in range(T):
            ps_o = mmpsum.tile([P, D], F32, tag="ps")
            ps_z = mmpsum.tile([P, 8], F32, tag="ps")
            for c in range(2):
                for j in range(T):
                    nc.tensor.matmul(
                        ps_o[:, c * 512 : (c + 1) * 512],
                        est[:, j * D + i * P : j * D + (i + 1) * P],
                        xb[:, j * XB_STRIDE + c * 512 : j * XB_STRIDE + (c + 1) * 512],
                        start=(j == 0),
                        stop=(j == T - 1),
                    )
            for j in range(T):
                nc.tensor.matmul(
                    ps_z[:, 0:8],
                    est[:, j * D + i * P : j * D + (i + 1) * P],
                    xb[:, j * XB_STRIDE + D : j * XB_STRIDE + D + 8],
                    start=(j == 0),
                    stop=(j == T - 1),
                )
            zi = stats.tile([P, 1], F32, tag="zi")
            nc.vector.tensor_copy(zi[:, :], ps_z[:, 0:1])
            rzi = stats.tile([P, 1], F32, tag="rzi")
            nc.vector.reciprocal(rzi[:, :], zi[:, :])
            outf = outpool.tile([P, D], F32, tag="of")
            nc.vector.tensor_scalar_mul(outf[:, :], ps_o[:, :], rzi[:, :])
            nc.sync.dma_start(out_ap[b, i * P : (i + 1) * P, :], outf[:, :])


_CACHED = {}


def _build():
    if "nc" in _CACHED:
        return _CACHED["nc"]
    nc = bacc.Bacc(
        "TRN2",
        target_bir_lowering=False,
        debug=False,
        enable_asserts=False,
        num_devices=N_CORES,
    )
    x_ap = nc.dram_tensor("x", [B_PER_CORE, N, D], F32, kind="ExternalInput").ap()
    out_ap = nc.dram_tensor(
        "out", [B_PER_CORE, N, D], F32, kind="ExternalOutput"
    ).ap()
    with tile.TileContext(nc) as tc:
        with ExitStack() as ctx:
            build_kernel_body(ctx, tc, x_ap, out_ap)
    nc.compile()
    _CACHED["nc"] = nc
    return nc


LAST_EXEC_NS = None


def kernel(x: np.ndarray) -> np.ndarray:
    global LAST_EXEC_NS
    x = np.ascontiguousarray(np.asarray(x, dtype=np.float32))
    B = x.shape[0]
    assert B == N_CORES * B_PER_CORE and x.shape[1:] == (N, D)
    nc = _build()
    shards = x.reshape(N_CORES, B_PER_CORE, N, D)
    in_maps = [{"x": np.ascontiguousarray(shards[i])} for i in range(N_CORES)]
    trace = os.environ.get("KL_TRACE", "0") == "1"
    res = run_bass_kernel_spmd(
        nc, in_maps, core_ids=list(range(N_CORES)), trace=trace
    )
    LAST_EXEC_NS = res.exec_time_ns
    out = np.concatenate([r["out"] for r in res.results], axis=0)
    return out.astype(np.float32, copy=False)
